# revision 6
# baseline (speedup 1.0000x reference)
"""Trainium2 Bass kernel for nn_HarmonicLayer (distance log-softmax loss).

Math (per reference):
    d[b,o]  = ||x_b||^2 + ||w_o||^2 - 2 x_b.w_o   (clamped at 1e-8; never
              binds for this data regime, d ~ 2048)
    s[b,o]  = -10 * ln(d[b,o])
    out     = s - logsumexp_o(s)

Sharding: vocab-parallel over 8 NeuronCores. Each core holds a
[V=6288]-column shard of the (padded) vocab, computes the local GEMM
-2*x.w via TensorE in bf16, forms u' = ln(d * 2^-11) (shifted so u' ~ 0
stores accurately in bf16), reduces local (min u', sum exp(-10(u'-min)))
stats, AllGathers the per-core stats, and applies the global log-softmax
correction locally. Host only shards/transposes inputs and concatenates
the per-core output columns.
"""

import sys

sys.path.insert(0, "/opt/trn_rl_repo")

import numpy as np
import ml_dtypes

from concourse import bacc, mybir, tile
from concourse.bass_utils import run_bass_kernel_spmd

N_CORES = 8
B, DIN, VOCAB = 2048, 1024, 50257
P = 128
V = 6288                 # per-core padded vocab columns (8*6288 = 50304)
VPAD = V * N_CORES
PAD_VAL = 1.0e6          # pad weight rows -> huge distance -> exp() == 0
KCH = DIN // P           # 8 contraction chunks
BT = B // P              # 16 batch tiles
GROUPS = [(0, 2048), (2048, 2048), (4096, 2048), (6144, 144)]
FGROUPS = [(0, 1572), (1572, 1572), (3144, 1572), (4716, 1572)]  # staging/fixup
SCALE_IN = float(2.0 ** -11)   # u' = ln(d * 2^-11) keeps u' near 0 for bf16

dt = mybir.dt
AF = mybir.ActivationFunctionType
ALU = mybir.AluOpType
AX = mybir.AxisListType


def build_body(nc, tc, x_d, xT_d, wT_d, wsq_d, out_d):
    fp32, bf16 = dt.float32, dt.bfloat16
    RG = [list(range(N_CORES))]
    with (
        tc.tile_pool(name="wres", bufs=1) as wres_p,
        tc.tile_pool(name="stg", bufs=3) as stg_p,
        tc.tile_pool(name="upool", bufs=3) as u_p,
        tc.tile_pool(name="epool", bufs=1) as e_p,
        tc.tile_pool(name="xpool", bufs=1) as x_p,
        tc.tile_pool(name="x2pool", bufs=1) as x2_p,
        tc.tile_pool(name="xtpool", bufs=2) as xt_p,
        tc.tile_pool(name="small", bufs=4) as sm_p,
        tc.tile_pool(name="psum", bufs=2, space="PSUM") as ps_p,
        tc.tile_pool(name="dram", bufs=4, space="DRAM") as dr_p,
    ):
        wt_all = wres_p.tile([P, KCH * V], bf16, name="wt_all")
        wsqb = wres_p.tile([P, V], bf16, name="wsqb")
        zero_ap = wres_p.tile([P, 1], fp32, name="zero_ap")
        nc.vector.memset(zero_ap[:], 0.0)

        # wsq load + cast to bf16 (stage via stg pool)
        for c0, cw in FGROUPS:
            st = stg_p.tile([P, 1572], fp32, tag="stg", name="st_wsq")
            nc.sync.dma_start(st[:, :cw], wsq_d[:, c0 : c0 + cw])
            nc.vector.tensor_copy(wsqb[:, c0 : c0 + cw], st[:, :cw])

        # wT load + cast: column-group outer so btile0/group0 can start early
        for c0, cw in FGROUPS:
            for k in range(KCH):
                st = stg_p.tile([P, 1572], fp32, tag="stg", name="st_wt")
                nc.sync.dma_start(
                    st[:, :cw], wT_d[k * P : (k + 1) * P, c0 : c0 + cw]
                )
                nc.vector.tensor_copy(
                    wt_all[:, k * V + c0 : k * V + c0 + cw], st[:, :cw]
                )

        xT_r = xT_d[:].rearrange("(k p) b -> p k b", p=P)

        for b in range(BT):
            bs = b * P
            # ||x_b||^2 per batch row (natural-layout x)
            xn = x_p.tile([P, DIN], fp32, tag="xn", name="xn")
            nc.sync.dma_start(xn[:], x_d[bs : bs + P, :])
            xn2 = x2_p.tile([P, DIN], fp32, tag="xn2", name="xn2")
            nc.vector.tensor_tensor(xn2[:], xn[:], xn[:], op=ALU.mult)
            xsq = sm_p.tile([P, 1], fp32, tag="xsq", name="xsq")
            nc.vector.tensor_reduce(xsq[:], xn2[:], axis=AX.X, op=ALU.add)
            xsqs = sm_p.tile([P, 1], fp32, tag="xsqs", name="xsqs")
            nc.vector.tensor_scalar(
                out=xsqs[:], in0=xsq[:], scalar1=SCALE_IN, scalar2=None,
                op0=ALU.mult,
            )

            # x^T stationary slice: [i=128 part, (k,b) free], scaled by -2
            xts = stg_p.tile([P, 1572], fp32, tag="stg", name="xts")
            nc.sync.dma_start(
                xts[:, : KCH * P].rearrange("p (k b) -> p k b", k=KCH),
                xT_r[:, :, bs : bs + P],
            )
            xtb = xt_p.tile([P, KCH * P], bf16, tag="xtb", name="xtb")
            nc.vector.tensor_scalar(
                out=xtb[:], in0=xts[:, : KCH * P], scalar1=-2.0, scalar2=None,
                op0=ALU.mult,
            )

            u_b = u_p.tile([P, V], bf16, tag="u", name="u_b")
            for c0, cw in GROUPS:
                ps = ps_p.tile([P, 2048], fp32, tag="ps", name="ps")
                for k in range(KCH):
                    for j0 in range(0, cw, 512):
                        jw = min(512, cw - j0)
                        nc.tensor.matmul(
                            ps[:, j0 : j0 + jw],
                            xtb[:, k * P : (k + 1) * P],
                            wt_all[:, k * V + c0 + j0 : k * V + c0 + j0 + jw],
                            start=(k == 0),
                            stop=(k == KCH - 1),
                        )
                # psum = -2*x.w ; += ||w||^2 ; then u' = ln(2^-11 d)
                nc.vector.tensor_tensor(
                    ps[:, :cw], ps[:, :cw], wsqb[:, c0 : c0 + cw], op=ALU.add
                )
                nc.scalar.activation(
                    u_b[:, c0 : c0 + cw], ps[:, :cw], AF.Ln,
                    bias=xsqs[:], scale=SCALE_IN,
                )

            # local stats: m = min u', S = sum exp(-10 u' + 10 m)
            m = sm_p.tile([P, 1], fp32, tag="m", name="m")
            nc.vector.tensor_reduce(m[:], u_b[:], axis=AX.X, op=ALU.min)
            tenm = sm_p.tile([P, 1], fp32, tag="tenm", name="tenm")
            nc.vector.tensor_scalar(
                out=tenm[:], in0=m[:], scalar1=10.0, scalar2=None, op0=ALU.mult
            )
            e_b = e_p.tile([P, V], bf16, tag="e", name="e_b")
            S = sm_p.tile([P, 1], fp32, tag="S", name="S")
            nc.scalar.activation(
                e_b[:], u_b[:], AF.Exp, bias=tenm[:], scale=-10.0,
                accum_out=S[:],
            )

            # AllGather per-core (m, S)
            stat2 = sm_p.tile([P, 2], fp32, tag="stat2", name="stat2")
            nc.vector.tensor_copy(stat2[:, 0:1], m[:])
            nc.vector.tensor_copy(stat2[:, 1:2], S[:])
            cc_in = dr_p.tile([P, 2], fp32, tag="ccin", name="cc_in")
            cc_out = dr_p.tile(
                [N_CORES * P, 2], fp32, tag="ccout", name="cc_out",
                addr_space="Shared",
            )
            nc.sync.dma_start(cc_in[:], stat2[:])
            nc.gpsimd.collective_compute(
                "AllGather", ALU.bypass, replica_groups=RG,
                ins=[cc_in[:]], outs=[cc_out[:]],
            )
            gth = sm_p.tile([P, N_CORES * 2], fp32, tag="gth", name="gth")
            nc.sync.dma_start(
                gth[:].rearrange("p (r s) -> p r s", s=2),
                cc_out[:].rearrange("(r p) s -> p r s", p=P),
            )
            gth3 = gth[:].rearrange("p (r s) -> p r s", s=2)
            ms_ap = gth3[:, :, 0]
            Ss_ap = gth3[:, :, 1]

            # global stats -> beta = 10*m_g - ln(S_g)
            mg = sm_p.tile([P, 1], fp32, tag="mg", name="mg")
            nc.vector.tensor_reduce(mg[:], ms_ap, axis=AX.X, op=ALU.min)
            tmg = sm_p.tile([P, 1], fp32, tag="tmg", name="tmg")
            nc.vector.tensor_scalar(
                out=tmg[:], in0=mg[:], scalar1=10.0, scalar2=None, op0=ALU.mult
            )
            ed = sm_p.tile([P, N_CORES], fp32, tag="ed", name="ed")
            nc.scalar.activation(ed[:], ms_ap, AF.Exp, bias=tmg[:], scale=-10.0)
            prod = sm_p.tile([P, N_CORES], fp32, tag="prod", name="prod")
            nc.vector.tensor_tensor(prod[:], ed[:], Ss_ap, op=ALU.mult)
            Sg = sm_p.tile([P, 1], fp32, tag="Sg", name="Sg")
            nc.vector.tensor_reduce(Sg[:], prod[:], axis=AX.X, op=ALU.add)
            lnS = sm_p.tile([P, 1], fp32, tag="lnS", name="lnS")
            nc.scalar.activation(lnS[:], Sg[:], AF.Ln, bias=zero_ap[:], scale=1.0)
            beta = sm_p.tile([P, 1], fp32, tag="beta", name="beta")
            nc.vector.tensor_scalar(
                out=beta[:], in0=lnS[:], scalar1=-1.0, scalar2=tmg[:],
                op0=ALU.mult, op1=ALU.add,
            )

            # fixup + store: y = -10*u' + beta
            for c0, cw in FGROUPS:
                y = stg_p.tile([P, 1572], fp32, tag="stg", name="y")
                nc.vector.tensor_scalar(
                    out=y[:, :cw], in0=u_b[:, c0 : c0 + cw], scalar1=-10.0,
                    scalar2=beta[:], op0=ALU.mult, op1=ALU.add,
                )
                nc.sync.dma_start(out_d[bs : bs + P, c0 : c0 + cw], y[:, :cw])


_NC_CACHE = None


def build_nc():
    global _NC_CACHE
    if _NC_CACHE is not None:
        return _NC_CACHE
    nc = bacc.Bacc(
        "TRN2", target_bir_lowering=False, debug=False, num_devices=N_CORES
    )
    x_d = nc.dram_tensor("x", [B, DIN], dt.float32, kind="ExternalInput")
    xT_d = nc.dram_tensor("xT", [DIN, B], dt.float32, kind="ExternalInput")
    wT_d = nc.dram_tensor("wT", [DIN, V], dt.float32, kind="ExternalInput")
    wsq_d = nc.dram_tensor("wsq", [P, V], dt.float32, kind="ExternalInput")
    out_d = nc.dram_tensor("out", [B, V], dt.float32, kind="ExternalOutput")
    with tile.TileContext(nc) as tc:
        build_body(nc, tc, x_d, xT_d, wT_d, wsq_d, out_d)
    nc.compile()
    _NC_CACHE = nc
    return nc


def make_in_maps(x, weight):
    x = np.ascontiguousarray(x, dtype=np.float32)
    weight = np.ascontiguousarray(weight, dtype=np.float32)
    w_pad = np.full((VPAD, DIN), PAD_VAL, dtype=np.float32)
    w_pad[:VOCAB] = weight
    xT = np.ascontiguousarray(x.T)
    in_maps = []
    for c in range(N_CORES):
        shard = w_pad[c * V : (c + 1) * V]
        wT = np.ascontiguousarray(shard.T)
        wb = shard.astype(ml_dtypes.bfloat16).astype(np.float32)
        wsq = np.einsum("vi,vi->v", wb, wb).astype(np.float32)
        wsq_rep = np.ascontiguousarray(
            np.broadcast_to(wsq[None, :], (P, V))
        )
        in_maps.append({"x": x, "xT": xT, "wT": wT, "wsq": wsq_rep})
    return in_maps


def kernel(x, weight):
    nc = build_nc()
    in_maps = make_in_maps(x, weight)
    res = run_bass_kernel_spmd(nc, in_maps, core_ids=list(range(N_CORES)))
    out = np.concatenate(
        [res.results[c]["out"] for c in range(N_CORES)], axis=1
    )[:, :VOCAB]
    return np.ascontiguousarray(out, dtype=np.float32)
